# revision 1
# baseline (speedup 1.0000x reference)
"""Multi-head causal self-attention (B=4, T=2048, C=768, H=12) on 8 trn2 cores.

Sharding: core c handles batch b = c//2 and head-group hg = c%2 (6 heads each).
Each core computes its QKV projection slice, causal attention for its 6 heads,
and a partial output projection (768x2048, transposed). Host sums the two
partials per batch, transposes back, and adds b_o. No cross-core collectives.

All on-chip compute uses a transposed data layout (feature dim on partitions,
token dim on the free axis) so no per-tile transposes are needed in the
attention inner loop; softmax denominators come from an appended ones-row in
the PV matmul; normalization happens after PV via a gpsimd partition
broadcast of the reciprocal denominator. Matmuls run as float32r (full-rate
fp32 mode on the PE; plain fp32 is 4x slower).

Emission order is engine-aware (engines execute in-order): QKV chunk groups
are interleaved with the head pairs they unblock, and within a head the
scores matmul for k-block kb+1 is emitted before the PV matmuls of k-block
kb so the PE never waits on the exp (ACT) of the current block.
"""

import math
import os

import numpy as np

import concourse.bass as bass
from concourse import bacc
import concourse.mybir as mybir
import concourse.tile as tile
from concourse import bass_utils
from concourse.bass import ts
from concourse.masks import make_identity

F32 = mybir.dt.float32
F32R = mybir.dt.float32r

P = 128
T = 2048          # sequence length
C = 768           # embed dim
CS = C // P       # 6 contraction chunks
HL = 6            # heads per core
HD = 64           # head dim
O = 3 * HL * HD   # 1152 rows of the local W_attn slice (q|k|v)
OB = O // P       # 9
J = HL * HD       # 384 local y-feature dim
JS = J // P       # 3
OUTB = C // P     # 6 output row blocks
TT = T // 512     # 4 column tiles of 512


def _build_bass():
    nc = bacc.Bacc("TRN2", target_bir_lowering=False, debug=False)
    x_d = nc.dram_tensor("x", [T, C], F32, kind="ExternalInput").ap()
    w_d = nc.dram_tensor("w", [O, C], F32, kind="ExternalInput").ap()
    b_d = nc.dram_tensor("b", [O], F32, kind="ExternalInput").ap()
    wo_d = nc.dram_tensor("wo", [C, J], F32, kind="ExternalInput").ap()
    out_d = nc.dram_tensor("out", [C, T], F32, kind="ExternalOutput").ap()

    with tile.TileContext(nc) as tc, nc.allow_low_precision(
        reason="fp32r matmul pipeline; fp32 PSUM accumulation throughout"
    ):
        _emit_kernel(tc, x_d, w_d, b_d, wo_d, out_d)
    nc.compile()
    return nc


def _emit_kernel(tc, x_d, w_d, b_d, wo_d, out_d):
    nc = tc.nc
    scale = 1.0 / math.sqrt(HD)

    x_r = x_d.rearrange("(tb p) c -> p tb c", p=P)      # [128, 16, 768]
    w_r = w_d.rearrange("(ob p) c -> p ob c", p=P)      # [128, 9, 768]
    wo_r = wo_d.rearrange("(ob p) j -> p ob j", p=P)    # [128, 6, 384]
    out_r = out_d.rearrange("(ob p) t -> p ob t", p=P)  # [128, 6, 2048]

    with (
        tc.tile_pool(name="persist", bufs=1) as persist,
        tc.tile_pool(name="stage", bufs=2) as stage,
        tc.tile_pool(name="attn", bufs=2) as attn,
        tc.tile_pool(name="ps512", bufs=2, space="PSUM") as ps512,
        tc.tile_pool(name="ps_s", bufs=2, space="PSUM") as ps_s,
        tc.tile_pool(name="ps_y", bufs=2, space="PSUM") as ps_y,
    ):
        ident = persist.tile([P, P], F32)
        make_identity(nc, ident)
        identr = persist.tile([P, P], F32R)
        nc.vector.tensor_copy(identr, ident)
        ones32 = persist.tile([P, HD], F32)
        nc.vector.memset(ones32, 1.0)
        ones1 = persist.tile([1, HD], F32R)
        nc.vector.tensor_copy(ones1, ones32[0:1, :])
        bsb = persist.tile([P, OB], F32)
        nc.sync.dma_start(bsb, b_d.rearrange("(a p) -> p a", p=P))

        xt = persist.tile([P, CS, T], F32R)      # x^T   48KB/partition
        wt = persist.tile([P, CS, O], F32R)      # W^T   27KB
        wot = persist.tile([P, JS, C], F32R)     # Wo^T   9KB
        qkvT = persist.tile([P, OB, T], F32R)    # qkv^T 72KB
        yT = persist.tile([P, JS, T], F32R)      # y^T   24KB

        def transpose_pack(src_tile, n_blk, dst_fn):
            """PE-transpose n_blk [128,128] column blocks of src_tile into a
            packed PSUM tile, then one ACT copy into dst via dst_fn(psum3d)."""
            pk = ps_s.tile([P, 1024], F32, tag="s")
            for i in range(n_blk):
                nc.tensor.transpose(pk[:, ts(i, P)], src_tile[:, ts(i, P)], ident)
            dst_fn(pk[:, : n_blk * P].rearrange("p (a b) -> p a b", b=P))

        ob_order = [0, 3, 6, 1, 4, 7, 2, 5, 8]

        def emit_w(ob):
            wn = stage.tile([P, C], F32, tag="ld", name="wn", bufs=3)
            nc.sync.dma_start(wn[:, : C // 2], w_r[:, ob, : C // 2])
            nc.sync.dma_start(wn[:, C // 2 :], w_r[:, ob, C // 2 :])
            transpose_pack(
                wn, CS, lambda pk, ob=ob: nc.scalar.copy(wt[:, :, ts(ob, P)], pk)
            )

        def emit_x(tb):
            xn = stage.tile([P, C], F32, tag="ld", name="xn", bufs=3)
            nc.sync.dma_start(xn[:, : C // 2], x_r[:, tb, : C // 2])
            nc.sync.dma_start(xn[:, C // 2 :], x_r[:, tb, C // 2 :])
            transpose_pack(
                xn, CS, lambda pk, tb=tb: nc.vector.tensor_copy(xt[:, :, ts(tb, P)], pk)
            )

        def emit_wo():
            for ob in range(OUTB):
                won = stage.tile([P, C], F32, tag="ld", name="won", bufs=3)[:, :J]
                nc.sync.dma_start(won, wo_r[:, ob, :])
                transpose_pack(
                    won, JS,
                    lambda pk, ob=ob: nc.scalar.copy(wot[:, :, ts(ob, P)], pk),
                )

        def emit_qkv(ob, tts=None):
            # qkv^T[o, t] = sum_c W^T[c, o] x^T[c, t] + b[o]
            for tt in (range(TT) if tts is None else tts):
                pq = ps512.tile([P, 512], F32, tag="mm")
                for cs in range(CS):
                    nc.tensor.matmul(
                        pq,
                        wt[:, cs, ts(ob, P)],
                        xt[:, cs, ts(tt, 512)],
                        start=(cs == 0),
                        stop=(cs == CS - 1),
                    )
                nc.vector.tensor_scalar_add(
                    qkvT[:, ob, ts(tt, 512)], pq, bsb[:, ob : ob + 1]
                )

        def emit_head(hl, mid_cb=None, late_cb=None):
            p0 = (hl % 2) * HD
            qT = qkvT[p0 : p0 + HD, hl // 2, :]       # [64, 2048] Q^T
            kT = qkvT[p0 : p0 + HD, 3 + hl // 2, :]   # [64, 2048] K^T
            vT = qkvT[p0 : p0 + HD, 6 + hl // 2, :]   # [64, 2048] V^T
            idd = identr[p0 : p0 + HD, p0 : p0 + HD]

            # V^T -> V (natural [k, d]) with an appended ones column
            vaug = attn.tile([P, T // P, HD + 1], F32R, tag="vaug", bufs=1)
            nc.vector.tensor_copy(
                vaug[:, :, HD : HD + 1], ones32[:, 0 : T // P, None]
            )
            for g in range(2):
                pk = ps512.tile([P, 512], F32R, tag="mm")
                for i in range(8):
                    nc.tensor.transpose(
                        pk[:, ts(i, HD)], vT[:, ts(g * 8 + i, P)], idd
                    )
                nc.vector.tensor_copy(
                    vaug[:, g * 8 : (g + 1) * 8, 0:HD],
                    pk.rearrange("p (a b) -> p a b", b=HD),
                )

            norm_q = []

            def flush_norms():
                while norm_q:
                    qt, yu, rd_t = norm_q.pop(0)
                    bc = ps512.tile([P, 512], F32, tag="mm", name="bc")
                    nc.tensor.matmul(
                        bc[0:HD], ones1[0:1], rd_t[0:1],
                        start=True, stop=True,
                    )
                    nc.vector.tensor_mul(
                        out=yT[p0 : p0 + HD, hl // 2, ts(qt, 512)],
                        in0=yu[0:HD],
                        in1=bc[0:HD],
                    )

            def emit_pv(kb, att, q0, lq, hf, ya_tiles):
                for qt in (2 * hf, 2 * hf + 1):
                    if kb > 4 * qt + 3:
                        continue
                    c0 = max(0, qt * 512 - q0)
                    c1 = min(lq, (qt + 1) * 512 - q0)
                    o0 = q0 + c0 - qt * 512
                    ya = ya_tiles[qt]
                    nc.tensor.matmul(
                        ya[0 : HD + 1, o0 : o0 + (c1 - c0)],
                        vaug[:, kb, :],
                        att[:, c0:c1],
                        start=(kb == 0),
                        stop=(kb == 4 * qt + 3),
                    )
                    if kb == 4 * qt + 3:
                        # normalization part A: pull y+denom out of PSUM and
                        # take the reciprocal; part B is deferred a k-block
                        # so the PE's broadcast matmul never waits on DVE
                        yu = attn.tile([P, 512], F32, tag="yu")
                        nc.vector.tensor_copy(yu[0 : HD + 1], ya[0 : HD + 1])
                        rd_t = attn.tile([P, 512], F32R, tag="yu", name="rd_t")
                        nc.vector.reciprocal(rd_t[0:1], yu[HD : HD + 1])
                        norm_q.append((qt, yu, rd_t))

            # flat (hf, kb) stage list: the scores/exp of stage i+1 are
            # emitted before the PV matmuls of stage i, across hf boundaries
            stages = [(0, kb) for kb in range(8)] + [(1, kb) for kb in range(16)]
            ya_tiles = {}
            pending = None
            for hf, kb in stages:
                if hf == 1 and kb == 14 and late_cb is not None:
                    late_cb()
                if hf == 1 and kb == 0 and mid_cb is not None:
                    # drain hf0 fully (PV + norms) before the callback reads yT
                    if pending is not None:
                        emit_pv(*pending, ya_tiles)
                        pending = None
                    flush_norms()
                    mid_cb()
                for qt in (2 * hf, 2 * hf + 1):
                    if qt not in ya_tiles:
                        ya_tiles[qt] = ps_y.tile(
                            [P, 512], F32, tag="y", name=f"ya{hl}_{qt}"
                        )
                q0 = max(kb * P, hf * 1024)
                lq = (hf + 1) * 1024 - q0
                # scores^T[k, q] for k-block kb, q in [q0, q0+lq)
                sp = ps_s.tile([P, 1024], F32, tag="s")
                for j in range(0, lq, 512):
                    f = min(512, lq - j)
                    nc.tensor.matmul(
                        sp[:, j : j + f],
                        kT[:, ts(kb, P)],
                        qT[:, q0 + j : q0 + j + f],
                        start=True,
                        stop=True,
                    )
                att = attn.tile([P, 1024], F32R, tag="att", bufs=2)
                nc.scalar.activation(
                    att[:, :lq], sp[:, :lq],
                    mybir.ActivationFunctionType.Exp, scale=scale,
                )
                if kb * P == q0:
                    # diagonal block: zero out k > q entries
                    nc.gpsimd.affine_select(
                        out=att[:, :P], in_=att[:, :P],
                        compare_op=mybir.AluOpType.is_ge,
                        fill=0.0, base=0, channel_multiplier=-1,
                        pattern=[[1, P]],
                    )
                flush_norms()
                if pending is not None:
                    emit_pv(*pending, ya_tiles)
                pending = (kb, att, q0, lq, 0 if q0 + lq <= 1024 else 1)
            emit_pv(*pending, ya_tiles)
            flush_norms()

        # ---- output projection: part^T[o, t] = sum_j Wo^T[j, o] y^T[j, t]
        def emit_outproj(tts):
          for tt in tts:
            for ob in range(OUTB):
                po = ps512.tile([P, 512], F32, tag="mm")
                for js in range(JS):
                    nc.tensor.matmul(
                        po,
                        wot[:, js, ts(ob, P)],
                        yT[:, js, ts(tt, 512)],
                        start=(js == 0),
                        stop=(js == JS - 1),
                    )
                osb = stage.tile([P, C], F32, tag="ld", name="osb", bufs=3)[:, :512]
                nc.vector.tensor_copy(osb, po)
                nc.sync.dma_start(
                    out_r[:, ob, ts(2 * tt, 256)], osb[:, 0:256]
                )
                nc.sync.dma_start(
                    out_r[:, ob, ts(2 * tt + 1, 256)], osb[:, 256:512]
                )
          return

        # ---- loads/transposes and QKV group 0, interleaved at tt granularity
        for ob in ob_order[0:3]:
            emit_w(ob)
        qkv_units = []  # (ob, tt) ready once tt's x-blocks are transposed
        for tb in range(T // P):
            emit_x(tb)
            if tb % 4 == 3:
                qkv_units += [(ob, tb // 4) for ob in ob_order[0:3]]
            # drain at most one unit per x-block once available, rest at end
            if qkv_units and tb >= 3:
                ob, tt = qkv_units.pop(0)
                emit_qkv(ob, tts=[tt])
        for ob, tt in qkv_units:
            emit_qkv(ob, tts=[tt])

        # ---- interleave remaining QKV chunk groups with head pairs
        for g in range(3):
            if g > 0:
                for ob in ob_order[3 * g : 3 * g + 3]:
                    emit_w(ob)
                    emit_qkv(ob)
            emit_head(2 * g)
            if g == 2:
                emit_wo()
                emit_head(
                    2 * g + 1,
                    mid_cb=lambda: emit_outproj([0, 1]),
                    late_cb=lambda: emit_outproj([2]),
                )
            else:
                emit_head(2 * g + 1)


        emit_outproj([3])


_NC_CACHE = None
LAST_RESULTS = None


def _get_nc():
    global _NC_CACHE
    if _NC_CACHE is None:
        _NC_CACHE = _build_bass()
    return _NC_CACHE


def kernel(x, W_attn, b_attn, W_o, b_o):
    global LAST_RESULTS
    x = np.asarray(x, np.float32)
    W_attn = np.asarray(W_attn, np.float32)
    b_attn = np.asarray(b_attn, np.float32)
    W_o = np.asarray(W_o, np.float32)
    b_o = np.asarray(b_o, np.float32)

    B = x.shape[0]
    in_maps = []
    for core in range(8):
        b, hg = divmod(core, 2)
        sl = slice(hg * J, (hg + 1) * J)
        w_l = np.concatenate(
            [W_attn[sl], W_attn[768 + hg * J : 768 + (hg + 1) * J],
             W_attn[1536 + hg * J : 1536 + (hg + 1) * J]], axis=0
        )
        b_l = np.concatenate(
            [b_attn[sl], b_attn[768 + hg * J : 768 + (hg + 1) * J],
             b_attn[1536 + hg * J : 1536 + (hg + 1) * J]], axis=0
        )
        in_maps.append({
            "x": np.ascontiguousarray(x[b]),
            "w": np.ascontiguousarray(w_l),
            "b": np.ascontiguousarray(b_l),
            "wo": np.ascontiguousarray(W_o[:, sl]),
        })

    nc = _get_nc()
    LAST_RESULTS = bass_utils.run_bass_kernel_spmd(
        nc, in_maps, core_ids=list(range(8)),
        trace=bool(int(os.environ.get("KERNEL_TRACE", "0"))),
    )
    parts = [r["out"] for r in LAST_RESULTS.results]

    out = np.empty((B, T, C), np.float32)
    for b in range(B):
        out[b] = (parts[2 * b] + parts[2 * b + 1]).T + b_o
    return out



# revision 46
# speedup vs baseline: 1.3576x; 1.3576x over previous
"""Multi-head causal self-attention (B=4, T=2048, C=768, H=12) on 8 trn2 cores.

Sharding: core c handles batch b = c//2 and head-group hg = c%2 (6 heads).
All matmul inputs are pre-transposed and pre-converted to bf16 on the host,
so the device does zero layout transposes:

  - q^T,k^T projections run in feature-major orientation (W^T stationary,
    x^T moving) producing qk^T[o, t] directly.
  - v projection swaps operands (x^T chunk stationary, Wv^T moving) producing
    v in natural [token, dim] layout - exactly what the PV matmul's
    stationary side needs. The v bias is folded into the host-side output
    bias (Wo @ b_v is a constant vector).
  - softmax denominators come from a ones-column appended to the v stationary
    tile; normalization multiplies by a PE-broadcast reciprocal.
  - exp runs on ACT (true exp) for most k-blocks and on DVE for a few via a
    Schraudolph bf16 fast-exp (scores*23.083 + 16250.5 -> int16, bitcast as
    bf16), keeping the ACT engine under the PE's per-head budget.

Single sequential PE stream: v-proj, qk-proj, 6 heads (two half-passes each,
scores for k-block kb+1 emitted before PV of kb), output projection.
"""

import math
import os

import ml_dtypes
import numpy as np

import concourse.bass as bass
from concourse import bacc
import concourse.mybir as mybir
import concourse.tile as tile
from concourse import bass_utils
from concourse.bass import ts
from concourse.masks import make_identity

F32 = mybir.dt.float32
BF16 = mybir.dt.bfloat16
I16 = mybir.dt.int16

P = 128
T = 2048
C = 768
CS = C // P        # 6 contraction chunks
HL = 6             # heads per core
HD = 64
J = HL * HD        # 384 local v/y features
JS = J // P        # 3
OUTB = C // P      # 6 output row blocks
NTB = T // P       # 16 token blocks
SCALE = 1.0 / math.sqrt(HD)
# Schraudolph bf16 fast-exp: bitcast<bf16>(int16(s*scale*128/ln2 + 16256-adj))
SCH_MUL = SCALE * 128.0 / math.log(2.0)
SCH_ADD = 16256.0 - 5.5
# (head-invariant) map of (hf, kb) exp units -> engine ("dve"/"pool" use the
# Schraudolph fast-exp; everything else runs true exp on ACT)
SCH_UNITS = {
    (1, 1): "dve", (1, 3): "dve", (1, 5): "dve", (1, 7): "dve",
}
SCH_LATE = SCH_UNITS
# k-block slots where one queued q/k projection unit is emitted as filler
FEED_SLOTS = {(0, 3), (1, 11)}


def _build_bass():
    nc = bacc.Bacc("TRN2", target_bir_lowering=False, debug=False)
    xt_d = nc.dram_tensor("xt", [C, T], BF16, kind="ExternalInput").ap()
    wqk_d = nc.dram_tensor("wqk", [C, 2 * J], BF16, kind="ExternalInput").ap()
    wv_d = nc.dram_tensor("wv", [C, J], BF16, kind="ExternalInput").ap()
    wo_d = nc.dram_tensor("wo", [J, C], BF16, kind="ExternalInput").ap()
    bqk_d = nc.dram_tensor("bqk", [2 * J], F32, kind="ExternalInput").ap()
    out_d = nc.dram_tensor("out", [C, T], BF16, kind="ExternalOutput").ap()

    with tile.TileContext(nc) as tc, nc.allow_low_precision(
        reason="bf16 matmul pipeline; fp32 PSUM accumulation throughout"
    ):
        _emit_kernel(tc, xt_d, wqk_d, wv_d, wo_d, bqk_d, out_d)
    nc.compile()
    return nc


def _emit_kernel(tc, xt_d, wqk_d, wv_d, wo_d, bqk_d, out_d):
    nc = tc.nc

    xt_r = xt_d.rearrange("(cs p) t -> p cs t", p=P)     # [128, 6, 2048]
    wqk_r = wqk_d.rearrange("(cs p) o -> p cs o", p=P)   # [128, 6, 768]
    wv_r = wv_d.rearrange("(cs p) j -> p cs j", p=P)     # [128, 6, 384]
    wo_r = wo_d.rearrange("(js p) o -> p js o", p=P)     # [128, 3, 768]
    out_r = out_d.rearrange("(ob p) t -> p ob t", p=P)   # [128, 6, 2048]

    with (
        tc.tile_pool(name="persist", bufs=1) as persist,
        tc.tile_pool(name="attn", bufs=3) as attn,
        tc.tile_pool(name="stage", bufs=4) as stage,
        tc.tile_pool(name="ps_mm", bufs=2, space="PSUM") as ps_mm,
        tc.tile_pool(name="ps_s", bufs=2, space="PSUM") as ps_s,
        tc.tile_pool(name="ps_y", bufs=2, space="PSUM") as ps_y,
    ):
        # ---- persistent SBUF
        ident = persist.tile([P, P], F32)
        make_identity(nc, ident)
        idb = persist.tile([P, P], BF16)
        nc.vector.tensor_copy(idb, ident)
        # maskB[c, q] = -1e30 where c > q, else 0; accumulated into diagonal
        # score blocks via idb-stationary matmul => causal mask pre-exp
        maskB = persist.tile([P, P], BF16)
        nc.vector.memset(maskB, -1e30)
        nc.gpsimd.affine_select(
            out=maskB, in_=maskB, compare_op=mybir.AluOpType.is_ge,
            fill=0.0, base=-1, channel_multiplier=1, pattern=[[-1, P]],
        )
        xt = persist.tile([P, CS, T], BF16)
        wqk = persist.tile([P, CS, 2 * J], BF16)
        wv = persist.tile([P, CS, J], BF16)
        wo = persist.tile([P, JS, C], BF16)
        bsb = persist.tile([P, CS], F32)
        qkT = persist.tile([P, CS, T], BF16)      # rows: ob 0-2 q, 3-5 k
        vsb = persist.tile([P, NTB, HL * (HD + 1)], BF16)  # per tb: 6x65
        yT = persist.tile([P, JS, T], BF16)
        ones1 = persist.tile([1, HD], BF16)

        nc.vector.memset(ones1, 1.0)
        # ones columns of the v stationary tiles (col 64 of each 65-block)
        vsb4 = vsb.rearrange("p t (h c) -> p t h c", h=HL)
        nc.gpsimd.memset(vsb4[:, :, :, HD : HD + 1], 1.0)

        # two hw-dge queues (SP, ACT); transfers are serial in issue order, so
        # order by first consumer. bsb (needed late) goes via the Pool swdge.
        nc.sync.dma_start(xt[:, :, 0:128], xt_r[:, :, 0:128])
        nc.scalar.dma_start(wv[:, 0:3, :], wv_r[:, 0:3, :])
        nc.scalar.dma_start(wv[:, 3:6, :], wv_r[:, 3:6, :])
        nc.gpsimd.dma_start(bsb, bqk_d.rearrange("(a p) -> p a", p=P))
        nc.sync.dma_start(xt[:, :, 128:512], xt_r[:, :, 128:512])
        nc.scalar.dma_start(xt[:, :, 512:1024], xt_r[:, :, 512:1024])
        nc.scalar.dma_start(wqk[:, :, 0:128], wqk_r[:, :, 0:128])
        nc.scalar.dma_start(wqk[:, :, 384:512], wqk_r[:, :, 384:512])
        nc.sync.dma_start(xt[:, :, 1024:1536], xt_r[:, :, 1024:1536])
        nc.sync.dma_start(xt[:, :, 1536:2048], xt_r[:, :, 1536:2048])
        nc.scalar.dma_start(wqk[:, :, 128:384], wqk_r[:, :, 128:384])
        nc.scalar.dma_start(wqk[:, :, 512:768], wqk_r[:, :, 512:768])
        nc.sync.dma_start(wo, wo_r)
        # preload the ACT exp table while the PE is in the projection phase
        scratch = persist.tile([1, 1], F32)
        nc.scalar.activation(scratch, ones1[0:1, 0:1],
                             mybir.ActivationFunctionType.Exp, scale=0.125)

        # ---- phase 1: v projection (natural [token, dim] layout) chasing the
        # x DMA stream, with q/k projections (feature-major) interleaved as
        # each 512-token tile completes; head 0-1's q/k blocks first.
        def emit_v(tb):
            vps = ps_mm.tile([P, 512], F32, tag="mm")
            for cs in range(CS):
                nc.tensor.matmul(
                    vps[:, :J],
                    xt[:, cs, ts(tb, P)],
                    wv[:, cs, :],
                    start=(cs == 0),
                    stop=(cs == CS - 1),
                )
            nc.scalar.copy(
                vsb4[:, tb, :, 0:HD],
                vps[:, :J].rearrange("p (h c) -> p h c", c=HD),
            )

        def emit_qk(ob, tt):
            pq = ps_mm.tile([P, 512], F32, tag="mm")
            for cs in range(CS):
                nc.tensor.matmul(
                    pq,
                    wqk[:, cs, ts(ob, P)],
                    xt[:, cs, ts(tt, 512)],
                    start=(cs == 0),
                    stop=(cs == CS - 1),
                )
            nc.vector.tensor_scalar_add(
                qkT[:, ob, ts(tt, 512)], pq, bsb[:, ob : ob + 1]
            )

        for tb in range(8):
            emit_v(tb)
        emit_qk(0, 0)
        emit_qk(3, 0)
        for tb in range(8, 12):
            emit_v(tb)
        emit_qk(0, 1)
        emit_qk(3, 1)
        for tb in range(12, NTB):
            emit_v(tb)
        for tt in range(2, 4):
            emit_qk(0, tt)
            emit_qk(3, tt)

        # ---- phase 2: attention; q/k projection units for later heads are
        # fed into the low-PE-work k-block slots (hf0 and the hf1 tail) so
        # the exp/select pipeline latency is hidden behind independent work
        qk_feed = [(ob, tt) for ob in (1, 4, 2, 5) for tt in range(4)]

        def feed(h, hf, kb):
            if (hf, kb) in FEED_SLOTS and qk_feed:
                emit_qk(*qk_feed.pop(0))

        def drain_for_head(h):
            # hard precondition: the head's own q/k blocks must be projected
            need = {h // 2, 3 + h // 2}
            rest = [u for u in qk_feed if u[0] not in need]
            for u in qk_feed:
                if u[0] in need:
                    emit_qk(*u)
            qk_feed[:] = rest

        for h in range(HL):
            drain_for_head(h)
            p0 = (h % 2) * HD
            qT = qkT[p0 : p0 + HD, h // 2, :]
            kT = qkT[p0 : p0 + HD, 3 + h // 2, :]
            sch = SCH_UNITS if qk_feed else SCH_LATE
            _emit_head(tc, h, qT, kT, vsb, yT, idb, maskB,
                       ps_mm, ps_s, ps_y, attn,
                       sch=sch, kb_cb=lambda hf, kb, h=h: feed(h, hf, kb))

        # ---- phase 3: output projection; 4-deep psum rotation across two
        # pools, copies alternating ACT/DVE, stores alternating DMA queues
        n = 0
        for tt in range(4):
            for obg in range(2):
                last = tt == 3
                ost3 = stage.tile([P, 3, 512], BF16, tag="ost", bufs=3)
                for obi in range(3):
                    ob = obg * 3 + obi
                    pool = ps_mm if n % 2 == 0 else ps_y
                    po = pool.tile([P, 512], F32, tag="mm" if pool is ps_mm else "y")
                    for js in range(JS):
                        nc.tensor.matmul(
                            po,
                            wo[:, js, ts(ob, P)],
                            yT[:, js, ts(tt, 512)],
                            start=(js == 0),
                            stop=(js == JS - 1),
                        )
                    copy = (nc.vector.tensor_copy, nc.scalar.copy)[n % 2]
                    copy(ost3[:, obi, :], po)
                    if last:
                        # split the final stores so the tail transfer is short
                        q = nc.sync if obi % 2 == 0 else nc.scalar
                        q.dma_start(out_r[:, ob, ts(tt, 512)], ost3[:, obi, :])
                    n += 1
                if not last:
                    nc.sync.dma_start(
                        out_r[:, 3 * obg : 3 * obg + 3, ts(tt, 512)], ost3)


def _emit_head(tc, h, qT, kT, vsb, yT, idb, maskB, ps_mm, ps_s, ps_y, attn,
               sch=SCH_UNITS, kb_cb=None):
    nc = tc.nc
    p0 = (h % 2) * HD
    js = h // 2
    vstat = vsb.rearrange("p t (g c) -> p t g c", g=HL)[:, :, h, :]  # [128,16,65]

    norm_q = []

    def start_norm(qt, ya):
        # the whole normalize chain runs off-PE: DVE reciprocal, Pool
        # partition-broadcast, DVE multiply (one PSUM operand max per op)
        rd = attn.tile([1, 512], BF16, tag="rd", bufs=2)
        nc.vector.reciprocal(rd, ya[HD : HD + 1, :])
        bcs = attn.tile([HD, 512], BF16, tag="bcs", bufs=2)
        nc.gpsimd.partition_broadcast(bcs, rd)
        norm_q.append((qt, ya, bcs))

    def flush_norms():
        while norm_q:
            qt, ya, bcs = norm_q.pop(0)
            nc.vector.tensor_mul(
                out=yT[p0 : p0 + HD, js, ts(qt, 512)],
                in0=ya[0:HD, :],
                in1=bcs,
            )

    for hf in range(2):
        ya = {
            qt: ps_y.tile([P, 512], F32, tag="y", name=f"ya{qt % 2}")
            for qt in (2 * hf, 2 * hf + 1)
        }
        kbs = range(8 * (hf + 1))
        pending = None

        def emit_pv(kb, att_b, q0, cols):
            for qt in (2 * hf, 2 * hf + 1):
                if kb > 4 * qt + 3:
                    continue
                c0 = max(0, qt * 512 - q0)
                c1 = min(cols, (qt + 1) * 512 - q0)
                o0 = q0 + c0 - qt * 512
                nc.tensor.matmul(
                    ya[qt][0 : HD + 1, o0 : o0 + (c1 - c0)],
                    vstat[:, kb, :],
                    att_b[:, c0:c1],
                    start=(kb == 0),
                    stop=(kb == 4 * qt + 3),
                )
                if kb == 4 * qt + 3:
                    start_norm(qt, ya[qt])

        for kb in kbs:
            if kb_cb is not None:
                kb_cb(hf, kb)
            q0 = max(kb * P, hf * 1024)
            cols = (hf + 1) * 1024 - q0
            diag = kb * P == q0
            sp = ps_s.tile([P, 1024], F32, tag="s")
            for j in range(0, cols, 512):
                f = min(512, cols - j)
                nc.tensor.matmul(
                    sp[:, j : j + f],
                    kT[:, ts(kb, P)],
                    qT[:, q0 + j : q0 + j + f],
                    start=True,
                    stop=not (diag and j == 0),
                    skip_group_check=diag and j == 0,
                )
                if diag and j == 0:
                    # accumulate -1e30 into the strict upper triangle of the
                    # diagonal block: causal mask applied pre-exp on the PE
                    nc.tensor.matmul(
                        sp[:, 0:P], idb, maskB,
                        start=False, stop=True, skip_group_check=True,
                    )
            att_i = attn.tile([P, 1024], I16, tag="att", bufs=4)
            att_b = att_i.bitcast(BF16)
            eng = sch.get((hf, kb))
            if eng is not None:
                assert not diag, "Schraudolph saturation on masked cols unverified"
                e = nc.vector if eng == "dve" else nc.gpsimd
                e.tensor_scalar(
                    out=att_i[:, :cols], in0=sp[:, :cols],
                    scalar1=SCH_MUL, scalar2=SCH_ADD,
                    op0=mybir.AluOpType.mult, op1=mybir.AluOpType.add,
                )
            else:
                nc.scalar.activation(
                    att_b[:, :cols], sp[:, :cols],
                    mybir.ActivationFunctionType.Exp, scale=SCALE,
                )
            flush_norms()
            if pending is not None:
                emit_pv(*pending)
            pending = (kb, att_b, q0, cols)
        emit_pv(*pending)
    flush_norms()


_NC_CACHE = None
LAST_RESULTS = None


def _get_nc():
    global _NC_CACHE
    if _NC_CACHE is None:
        _NC_CACHE = _build_bass()
    return _NC_CACHE


def kernel(x, W_attn, b_attn, W_o, b_o):
    global LAST_RESULTS
    x = np.asarray(x, np.float32)
    W_attn = np.asarray(W_attn, np.float32)
    b_attn = np.asarray(b_attn, np.float32)
    W_o = np.asarray(W_o, np.float32)
    b_o = np.asarray(b_o, np.float32)
    BF = ml_dtypes.bfloat16

    B = x.shape[0]
    b_v = b_attn[2 * C:]
    out_bias = b_o + W_o @ b_v  # v bias folded through the output projection

    in_maps = []
    for core in range(8):
        b, hg = divmod(core, 2)
        sl = slice(hg * J, (hg + 1) * J)
        wq = W_attn[sl]                        # [384, 768]
        wk = W_attn[C + hg * J : C + (hg + 1) * J]
        wv_ = W_attn[2 * C + hg * J : 2 * C + (hg + 1) * J]
        in_maps.append({
            "xt": np.ascontiguousarray(x[b].T).astype(BF),
            "wqk": np.ascontiguousarray(
                np.concatenate([wq, wk], axis=0).T).astype(BF),
            "wv": np.ascontiguousarray(wv_.T).astype(BF),
            "wo": np.ascontiguousarray(W_o[:, sl].T).astype(BF),
            "bqk": np.ascontiguousarray(
                np.concatenate([b_attn[sl], b_attn[C + hg * J : C + (hg + 1) * J]])
            ),
        })

    nc = _get_nc()
    LAST_RESULTS = bass_utils.run_bass_kernel_spmd(
        nc, in_maps, core_ids=list(range(8)),
        trace=bool(int(os.environ.get("KERNEL_TRACE", "0"))),
    )
    parts = [r["out"] for r in LAST_RESULTS.results]

    out = np.empty((B, T, C), np.float32)
    for b in range(B):
        acc = parts[2 * b].astype(np.float32) + parts[2 * b + 1].astype(np.float32)
        out[b] = acc.T + out_bias
    return out


# revision 65
# speedup vs baseline: 1.3640x; 1.0047x over previous
"""Multi-head causal self-attention (B=4, T=2048, C=768, H=12) on 8 trn2 cores.

Sharding: core c handles batch b = c//2 and head-group hg = c%2 (6 heads).
All matmul inputs are pre-transposed and pre-converted to bf16 on the host,
so the device does zero layout transposes:

  - q^T,k^T projections run in feature-major orientation (W^T stationary,
    x^T moving) producing qk^T[o, t] directly.
  - v projection swaps operands (x^T chunk stationary, Wv^T moving) producing
    v in natural [token, dim] layout - exactly what the PV matmul's
    stationary side needs. The v bias is folded into the host-side output
    bias (Wo @ b_v is a constant vector).
  - the causal mask is applied on the PE: one extra 128-col matmul per
    diagonal score block accumulates -1e30 into the strict upper triangle of
    the PSUM scores before exp (no gpsimd affine_select in the inner loop).
  - softmax denominators come from a ones-column appended to the v stationary
    tile; the normalize chain runs entirely off-PE (DVE reciprocal -> Pool
    partition_broadcast -> DVE multiply, max one PSUM operand per op).
  - exp runs on ACT (true exp) for most k-blocks and on DVE for a few via a
    Schraudolph bf16 fast-exp (scores*23.083 + 16250.5 -> int16, bitcast as
    bf16), keeping the ACT engine under the PE's per-head budget.

Single sequential PE stream: v-proj, qk-proj, 6 heads (two half-passes each,
scores for k-block kb+1 emitted before PV of kb, queued q/k projection units
for later heads fed into low-PE-work k-block slots), output projection.
"""

import math
import os

import ml_dtypes
import numpy as np

import concourse.bass as bass
from concourse import bacc
import concourse.mybir as mybir
import concourse.tile as tile
from concourse import bass_utils
from concourse.bass import ts
from concourse.masks import make_identity

F32 = mybir.dt.float32
BF16 = mybir.dt.bfloat16
I16 = mybir.dt.int16

P = 128
T = 2048
C = 768
CS = C // P        # 6 contraction chunks
HL = 6             # heads per core
HD = 64
J = HL * HD        # 384 local v/y features
JS = J // P        # 3
OUTB = C // P      # 6 output row blocks
NTB = T // P       # 16 token blocks
SCALE = 1.0 / math.sqrt(HD)
# Schraudolph bf16 fast-exp: bitcast<bf16>(int16(s*scale*128/ln2 + 16256-adj))
SCH_MUL = SCALE * 128.0 / math.log(2.0)
SCH_ADD = 16256.0 - 5.5
# (head-invariant) map of (hf, kb) exp units -> engine ("dve"/"pool" use the
# Schraudolph fast-exp; everything else runs true exp on ACT)
SCH_UNITS = {
    (1, 1): "dve", (1, 3): "dve", (1, 5): "dve", (1, 7): "dve",
}
SCH_LATE = SCH_UNITS
# k-block slots where one queued q/k projection unit is emitted as filler
FEED_SLOTS = {(0, 3), (1, 11)}


def _build_bass():
    nc = bacc.Bacc("TRN2", target_bir_lowering=False, debug=False)
    xt_d = nc.dram_tensor("xt", [C, T], BF16, kind="ExternalInput").ap()
    wqk_d = nc.dram_tensor("wqk", [C, 2 * J], BF16, kind="ExternalInput").ap()
    wv_d = nc.dram_tensor("wv", [C, J], BF16, kind="ExternalInput").ap()
    wo_d = nc.dram_tensor("wo", [J, C], BF16, kind="ExternalInput").ap()
    bqk_d = nc.dram_tensor("bqk", [2 * J], F32, kind="ExternalInput").ap()
    out_d = nc.dram_tensor("out", [C, T], BF16, kind="ExternalOutput").ap()

    with tile.TileContext(nc) as tc, nc.allow_low_precision(
        reason="bf16 matmul pipeline; fp32 PSUM accumulation throughout"
    ):
        _emit_kernel(tc, xt_d, wqk_d, wv_d, wo_d, bqk_d, out_d)
    nc.compile()
    return nc


def _emit_kernel(tc, xt_d, wqk_d, wv_d, wo_d, bqk_d, out_d):
    nc = tc.nc

    xt_r = xt_d.rearrange("(cs p) t -> p cs t", p=P)     # [128, 6, 2048]
    wqk_r = wqk_d.rearrange("(cs p) o -> p cs o", p=P)   # [128, 6, 768]
    wv_r = wv_d.rearrange("(cs p) j -> p cs j", p=P)     # [128, 6, 384]
    wo_r = wo_d.rearrange("(js p) o -> p js o", p=P)     # [128, 3, 768]
    out_r = out_d.rearrange("(ob p) t -> p ob t", p=P)   # [128, 6, 2048]

    with (
        tc.tile_pool(name="persist", bufs=1) as persist,
        tc.tile_pool(name="attn", bufs=3) as attn,
        tc.tile_pool(name="stage", bufs=4) as stage,
        tc.tile_pool(name="ps_mm", bufs=2, space="PSUM") as ps_mm,
        tc.tile_pool(name="ps_s", bufs=2, space="PSUM") as ps_s,
        tc.tile_pool(name="ps_y", bufs=2, space="PSUM") as ps_y,
    ):
        # ---- persistent SBUF
        ident = persist.tile([P, P], F32)
        make_identity(nc, ident)
        idb = persist.tile([P, P], BF16)
        nc.vector.tensor_copy(idb, ident)
        # maskB[c, q] = -1e30 where c > q, else 0; accumulated into diagonal
        # score blocks via idb-stationary matmul => causal mask pre-exp
        # -500 (not -inf): masked entries give exp ~ e^-59 ~ 0, and the
        # Schraudolph int16 value stays positive-small (bf16 ~ 1e-27), so the
        # fast-exp path is safe on masked columns too
        maskB = persist.tile([P, P], BF16)
        nc.vector.memset(maskB, -500.0)
        nc.gpsimd.affine_select(
            out=maskB, in_=maskB, compare_op=mybir.AluOpType.is_ge,
            fill=0.0, base=-1, channel_multiplier=1, pattern=[[-1, P]],
        )
        xt = persist.tile([P, CS, T], BF16)
        wqk = persist.tile([P, CS, 2 * J], BF16)
        wv = persist.tile([P, CS, J], BF16)
        wo = persist.tile([P, JS, C], BF16)
        bsb = persist.tile([P, CS], F32)
        qkT = persist.tile([P, CS, T], BF16)      # rows: ob 0-2 q, 3-5 k
        vsb = persist.tile([P, NTB, HL * (HD + 1)], BF16)  # per tb: 6x65
        yT = persist.tile([P, JS, T], BF16)
        ones1 = persist.tile([1, HD], BF16)

        nc.vector.memset(ones1, 1.0)
        # ones columns of the v stationary tiles (col 64 of each 65-block)
        vsb4 = vsb.rearrange("p t (h c) -> p t h c", h=HL)
        nc.gpsimd.memset(vsb4[:, :, :, HD : HD + 1], 1.0)

        # two hw-dge queues (SP, ACT); transfers are serial in issue order, so
        # order by first consumer. bsb (needed late) goes via the Pool swdge.
        nc.sync.dma_start(xt[:, :, 0:128], xt_r[:, :, 0:128])
        nc.scalar.dma_start(wv[:, 0:3, :], wv_r[:, 0:3, :])
        nc.scalar.dma_start(wv[:, 3:6, :], wv_r[:, 3:6, :])
        nc.sync.dma_start(xt[:, :, 128:512], xt_r[:, :, 128:512])
        nc.gpsimd.dma_start(bsb, bqk_d.rearrange("(a p) -> p a", p=P))
        nc.scalar.dma_start(xt[:, :, 512:1024], xt_r[:, :, 512:1024])
        nc.scalar.dma_start(wqk[:, :, 0:128], wqk_r[:, :, 0:128])
        nc.scalar.dma_start(wqk[:, :, 384:512], wqk_r[:, :, 384:512])
        nc.sync.dma_start(xt[:, :, 1024:1536], xt_r[:, :, 1024:1536])
        nc.sync.dma_start(xt[:, :, 1536:2048], xt_r[:, :, 1536:2048])
        nc.scalar.dma_start(wqk[:, :, 128:384], wqk_r[:, :, 128:384])
        nc.scalar.dma_start(wqk[:, :, 512:768], wqk_r[:, :, 512:768])
        nc.sync.dma_start(wo, wo_r)
        # preload the ACT exp table while the PE is in the projection phase
        scratch = persist.tile([1, 1], F32)
        nc.scalar.activation(scratch, ones1[0:1, 0:1],
                             mybir.ActivationFunctionType.Exp, scale=0.125)

        # p-state warmup: keep the PE busy on the (DMA-free) identity tile
        # while the first x/wv transfers land, so real work starts at the
        # full 2.4GHz clock instead of ramping through the low p-states
        warm = ps_s.tile([P, 1024], F32, tag="s")
        for i in range(12):
            nc.tensor.matmul(warm[:, 0:P], ident, ident, start=True, stop=True)

        # ---- phase 1: v projection (natural [token, dim] layout) chasing the
        # x DMA stream, with q/k projections (feature-major) interleaved as
        # each 512-token tile completes; head 0-1's q/k blocks first.
        def emit_v(tb):
            vps = ps_mm.tile([P, 512], F32, tag="mm")
            for cs in range(CS):
                nc.tensor.matmul(
                    vps[:, :J],
                    xt[:, cs, ts(tb, P)],
                    wv[:, cs, :],
                    start=(cs == 0),
                    stop=(cs == CS - 1),
                )
            nc.scalar.copy(
                vsb4[:, tb, :, 0:HD],
                vps[:, :J].rearrange("p (h c) -> p h c", c=HD),
            )

        def emit_qk(ob, tt):
            pq = ps_mm.tile([P, 512], F32, tag="mm")
            for cs in range(CS):
                nc.tensor.matmul(
                    pq,
                    wqk[:, cs, ts(ob, P)],
                    xt[:, cs, ts(tt, 512)],
                    start=(cs == 0),
                    stop=(cs == CS - 1),
                )
            nc.vector.tensor_scalar_add(
                qkT[:, ob, ts(tt, 512)], pq, bsb[:, ob : ob + 1]
            )

        for tb in range(8):
            emit_v(tb)
        emit_qk(0, 0)
        emit_qk(3, 0)
        for tb in range(8, 12):
            emit_v(tb)
        emit_qk(0, 1)
        emit_qk(3, 1)
        for tb in range(12, NTB):
            emit_v(tb)
        for tt in range(2, 4):
            emit_qk(0, tt)
            emit_qk(3, tt)

        # ---- phase 2: attention; q/k projection units for later heads are
        # fed into the low-PE-work k-block slots (hf0 and the hf1 tail) so
        # the exp/select pipeline latency is hidden behind independent work
        qk_feed = [(ob, tt) for ob in (1, 4, 2, 5) for tt in range(4)]

        def feed(h, hf, kb):
            if (hf, kb) in FEED_SLOTS and qk_feed:
                emit_qk(*qk_feed.pop(0))

        def drain_for_head(h):
            # hard precondition: the head's own q/k blocks must be projected
            need = {h // 2, 3 + h // 2}
            rest = [u for u in qk_feed if u[0] not in need]
            for u in qk_feed:
                if u[0] in need:
                    emit_qk(*u)
            qk_feed[:] = rest

        # output-projection units for yT column-tiles 0-1 (ready after head
        # 5's first half-pass) are fed into head 5's hf1 k-block slots
        op_feed = [(ob, tt) for tt in (0, 1) for ob in range(OUTB)]

        def emit_op(ob, tt):
            po = ps_mm.tile([P, 512], F32, tag="mm")
            for js in range(JS):
                nc.tensor.matmul(
                    po,
                    wo[:, js, ts(ob, P)],
                    yT[:, js, ts(tt, 512)],
                    start=(js == 0),
                    stop=(js == JS - 1),
                )
            ost = stage.tile([P, 512], BF16, tag="ost1", bufs=2)
            nc.vector.tensor_copy(ost, po)
            nc.sync.dma_start(out_r[:, ob, ts(tt, 512)], ost)

        def feed_op(hf, kb):
            if hf == 1 and 2 <= kb <= 13 and op_feed:
                emit_op(*op_feed.pop(0))

        for h in range(HL):
            drain_for_head(h)
            p0 = (h % 2) * HD
            qT = qkT[p0 : p0 + HD, h // 2, :]
            kT = qkT[p0 : p0 + HD, 3 + h // 2, :]
            sch = SCH_UNITS if qk_feed else SCH_LATE
            cb = (lambda hf, kb, h=h: feed(h, hf, kb)) if h < HL - 1 else feed_op
            _emit_head(tc, h, qT, kT, vsb, yT, idb, maskB,
                       ps_mm, ps_s, ps_y, attn,
                       sch=sch, kb_cb=cb)

        # ---- phase 3: output projection; 4-deep psum rotation across two
        # pools, copies alternating ACT/DVE, stores alternating DMA queues
        for u in op_feed:
            emit_op(*u)
        n = 0
        for tt in range(2, 4):
            for obg in range(2):
                last = tt == 3
                ost3 = stage.tile([P, 3, 512], BF16, tag="ost", bufs=4)
                for obi in range(3):
                    ob = obg * 3 + obi
                    pool = ps_mm if n % 2 == 0 else ps_y
                    po = pool.tile([P, 512], F32, tag="mm" if pool is ps_mm else "y")
                    for js in range(JS):
                        nc.tensor.matmul(
                            po,
                            wo[:, js, ts(ob, P)],
                            yT[:, js, ts(tt, 512)],
                            start=(js == 0),
                            stop=(js == JS - 1),
                        )
                    copy = (nc.vector.tensor_copy, nc.scalar.copy)[n % 2]
                    copy(ost3[:, obi, :], po)
                    if last:
                        # split the final stores so the tail transfer is short
                        q = nc.sync if obi % 2 == 0 else nc.scalar
                        q.dma_start(out_r[:, ob, ts(tt, 512)], ost3[:, obi, :])
                    n += 1
                if not last:
                    nc.sync.dma_start(
                        out_r[:, 3 * obg : 3 * obg + 3, ts(tt, 512)], ost3)


def _emit_head(tc, h, qT, kT, vsb, yT, idb, maskB, ps_mm, ps_s, ps_y, attn,
               sch=SCH_UNITS, kb_cb=None):
    nc = tc.nc
    p0 = (h % 2) * HD
    js = h // 2
    vstat = vsb.rearrange("p t (g c) -> p t g c", g=HL)[:, :, h, :]  # [128,16,65]

    norm_q = []

    def start_norm(qt, ya):
        # the whole normalize chain runs off-PE: DVE reciprocal, Pool
        # partition-broadcast, DVE multiply (one PSUM operand max per op)
        rd = attn.tile([1, 512], BF16, tag="rd", bufs=2)
        nc.vector.reciprocal(rd, ya[HD : HD + 1, :])
        bcs = attn.tile([HD, 512], BF16, tag="bcs", bufs=2)
        nc.gpsimd.partition_broadcast(bcs, rd)
        norm_q.append((qt, ya, bcs))

    def flush_norms():
        while norm_q:
            qt, ya, bcs = norm_q.pop(0)
            nc.vector.tensor_mul(
                out=yT[p0 : p0 + HD, js, ts(qt, 512)],
                in0=ya[0:HD, :],
                in1=bcs,
            )

    for hf in range(2):
        ya = {
            qt: ps_y.tile([P, 512], F32, tag="y", name=f"ya{qt % 2}")
            for qt in (2 * hf, 2 * hf + 1)
        }
        kbs = range(8 * (hf + 1))
        pending = None

        def emit_pv(kb, att_b, q0, cols):
            for qt in (2 * hf, 2 * hf + 1):
                if kb > 4 * qt + 3:
                    continue
                c0 = max(0, qt * 512 - q0)
                c1 = min(cols, (qt + 1) * 512 - q0)
                o0 = q0 + c0 - qt * 512
                nc.tensor.matmul(
                    ya[qt][0 : HD + 1, o0 : o0 + (c1 - c0)],
                    vstat[:, kb, :],
                    att_b[:, c0:c1],
                    start=(kb == 0),
                    stop=(kb == 4 * qt + 3),
                )
                if kb == 4 * qt + 3:
                    start_norm(qt, ya[qt])

        for kb in kbs:
            if kb_cb is not None:
                kb_cb(hf, kb)
            q0 = max(kb * P, hf * 1024)
            cols = (hf + 1) * 1024 - q0
            diag = kb * P == q0
            sp = ps_s.tile([P, 1024], F32, tag="s")
            for j in range(0, cols, 512):
                f = min(512, cols - j)
                nc.tensor.matmul(
                    sp[:, j : j + f],
                    kT[:, ts(kb, P)],
                    qT[:, q0 + j : q0 + j + f],
                    start=True,
                    stop=not (diag and j == 0),
                    skip_group_check=diag and j == 0,
                )
                if diag and j == 0:
                    # accumulate -1e30 into the strict upper triangle of the
                    # diagonal block: causal mask applied pre-exp on the PE
                    nc.tensor.matmul(
                        sp[:, 0:P], idb, maskB,
                        start=False, stop=True, skip_group_check=True,
                    )
            att_i = attn.tile([P, 1024], I16, tag="att", bufs=6)
            att_b = att_i.bitcast(BF16)
            eng = sch.get((hf, kb))
            if eng is not None:
                e = nc.vector if eng == "dve" else nc.gpsimd
                e.tensor_scalar(
                    out=att_i[:, :cols], in0=sp[:, :cols],
                    scalar1=SCH_MUL, scalar2=SCH_ADD,
                    op0=mybir.AluOpType.mult, op1=mybir.AluOpType.add,
                )
            else:
                nc.scalar.activation(
                    att_b[:, :cols], sp[:, :cols],
                    mybir.ActivationFunctionType.Exp, scale=SCALE,
                )
            flush_norms()
            if pending is not None:
                emit_pv(*pending)
            pending = (kb, att_b, q0, cols)
        emit_pv(*pending)
    flush_norms()


_NC_CACHE = None
LAST_RESULTS = None


def _get_nc():
    global _NC_CACHE
    if _NC_CACHE is None:
        _NC_CACHE = _build_bass()
    return _NC_CACHE


def kernel(x, W_attn, b_attn, W_o, b_o):
    global LAST_RESULTS
    x = np.asarray(x, np.float32)
    W_attn = np.asarray(W_attn, np.float32)
    b_attn = np.asarray(b_attn, np.float32)
    W_o = np.asarray(W_o, np.float32)
    b_o = np.asarray(b_o, np.float32)
    BF = ml_dtypes.bfloat16

    B = x.shape[0]
    b_v = b_attn[2 * C:]
    out_bias = b_o + W_o @ b_v  # v bias folded through the output projection

    in_maps = []
    for core in range(8):
        b, hg = divmod(core, 2)
        sl = slice(hg * J, (hg + 1) * J)
        wq = W_attn[sl]                        # [384, 768]
        wk = W_attn[C + hg * J : C + (hg + 1) * J]
        wv_ = W_attn[2 * C + hg * J : 2 * C + (hg + 1) * J]
        in_maps.append({
            "xt": np.ascontiguousarray(x[b].T).astype(BF),
            "wqk": np.ascontiguousarray(
                np.concatenate([wq, wk], axis=0).T).astype(BF),
            "wv": np.ascontiguousarray(wv_.T).astype(BF),
            "wo": np.ascontiguousarray(W_o[:, sl].T).astype(BF),
            "bqk": np.ascontiguousarray(
                np.concatenate([b_attn[sl], b_attn[C + hg * J : C + (hg + 1) * J]])
            ),
        })

    nc = _get_nc()
    LAST_RESULTS = bass_utils.run_bass_kernel_spmd(
        nc, in_maps, core_ids=list(range(8)),
        trace=bool(int(os.environ.get("KERNEL_TRACE", "0"))),
    )
    parts = [r["out"] for r in LAST_RESULTS.results]

    out = np.empty((B, T, C), np.float32)
    for b in range(B):
        acc = parts[2 * b].astype(np.float32) + parts[2 * b + 1].astype(np.float32)
        out[b] = acc.T + out_bias
    return out
